# revision 14
# baseline (speedup 1.0000x reference)
"""Distributed causal self-attention on 8 TRN2 NeuronCores.

Strategy (Megatron-style tensor parallel on heads + token-split output):
  - Each core owns 2 of the 16 heads. It computes qkv projection for its
    heads (full batch/seq), causal attention (transposed-score layout so
    softmax sums come free from an appended ones-row on V), producing
    y[heads_local, tokens].
  - AllToAll reshards y from head-split to token-split: afterwards each
    core holds all 1024 features for its 1024-token slice.
  - Each core computes its token slice of the output projection.
  - Host gather is pure concatenation along tokens.

Layouts: everything TensorEngine-facing keeps the contraction dim on
partitions. x is passed pre-transposed [D, B*T]; qkv weights pre-sliced
and transposed per core [D, 384]; w_out pre-transposed [feat, D].
"""

import os
import sys

sys.path.insert(0, "/opt/trn_rl_repo")

import numpy as np

import concourse.bass as bass
import concourse.mybir as mybir
import concourse.tile as tile
from concourse import bacc
from concourse.bass_utils import run_bass_kernel_spmd
from concourse.masks import make_identity

def _install_profile_hook():
    """The RL container's antenv stub lacks axon_hooks, so bass_utils can't
    reach the NTFF profiler. Recreate the tiny set/get module and wire it to
    trn_boot's ctypes hook against libaxon_pjrt.so."""
    import types

    if "antenv.axon_hooks" in sys.modules:
        return
    try:
        import antenv
        from trn_agent_boot.trn_boot import _ntff_profile_via_ctypes

        mod = types.ModuleType("antenv.axon_hooks")
        mod._hook = None

        def set_axon_ntff_profile_hook(h):
            mod._hook = h

        def get_axon_ntff_profile_hook():
            return mod._hook

        mod.set_axon_ntff_profile_hook = set_axon_ntff_profile_hook
        mod.get_axon_ntff_profile_hook = get_axon_ntff_profile_hook
        sys.modules["antenv.axon_hooks"] = mod
        antenv.axon_hooks = mod
        hook = _ntff_profile_via_ctypes("/opt/axon/libaxon_pjrt.so")
        if hook is not None:
            mod._hook = hook
    except Exception as e:  # profiling is best-effort; execution must work
        print(f"profile hook install failed: {e}", file=sys.stderr)


B, T, D, H, DH = 4, 2048, 1024, 16, 64
BT = B * T              # 8192 tokens
N_CORES = 8
HL = H // N_CORES       # 2 heads per core
FL = HL * DH            # 128 local features
TSLICE = BT // N_CORES  # 1024 tokens per core in output
SCALE = DH ** -0.5
F32 = mybir.dt.float32
# TensorEngine compute dtype. float32r is the same 4-byte memory format but
# runs 1.5 cycles/row on the PE instead of fp32's 2.
USE_F32R = os.environ.get("ATTN_F32R", "1") != "0"
F32R = mybir.dt.float32r


MMDT = F32R if USE_F32R else F32
BF16 = mybir.dt.bfloat16

IB = 512     # query block (free dim of transposed score matmuls)
NJ = T // 128  # 16 key tiles per (b, h)


def _build():
    nc = bacc.Bacc("TRN2", target_bir_lowering=False, debug=False,
                   num_devices=N_CORES)

    xT = nc.dram_tensor("xT", [D, BT], MMDT, kind="ExternalInput")
    wqkvT = nc.dram_tensor("wqkvT", [D, 3 * FL], MMDT,
                           kind="ExternalInput")
    woutT = nc.dram_tensor("woutT", [D, D], MMDT, kind="ExternalInput")
    out = nc.dram_tensor("out", [TSLICE, D], F32, kind="ExternalOutput")

    xT_r = xT[:].rearrange("(o p) t -> p o t", p=128)        # [128, 8, BT]
    wqkvT_r = wqkvT[:].rearrange("(o p) f -> p o f", p=128)  # [128, 8, 384]
    woutT_r = woutT[:].rearrange("(o p) d -> p o d", p=128)  # [128, 8, 1024]

    with tile.TileContext(nc) as tc:
        from contextlib import ExitStack

        with ExitStack() as ctx:
            const = ctx.enter_context(tc.tile_pool(name="const", bufs=1))
            wpool = ctx.enter_context(tc.tile_pool(name="wpool", bufs=1))
            xpool = ctx.enter_context(tc.tile_pool(name="xpool", bufs=2))
            qkvpool = ctx.enter_context(tc.tile_pool(name="qkvpool", bufs=2))
            yun = ctx.enter_context(tc.tile_pool(name="yun", bufs=10))
            vpool = ctx.enter_context(tc.tile_pool(name="vpool", bufs=3))
            ppool = ctx.enter_context(tc.tile_pool(name="ppool", bufs=22))
            ypool = ctx.enter_context(tc.tile_pool(name="ypool", bufs=3))
            opool = ctx.enter_context(tc.tile_pool(name="opool", bufs=2))
            psA = ctx.enter_context(
                tc.tile_pool(name="psA", bufs=5, space="PSUM"))
            psY = ctx.enter_context(
                tc.tile_pool(name="psY", bufs=2, space="PSUM"))
            dram = ctx.enter_context(
                tc.tile_pool(name="dram", bufs=1, space="DRAM"))

            # ---- constants ----
            identity = const.tile([128, 128], F32, tag="identity")
            make_identity(nc, identity[:])
            # masks[oi][p, f] = 1.0 where f - p - oi*128 >= 0 else 0
            # (keep key j0+p for query i0+f iff j <= i, offset oi*128 = j0-i0)
            masks = []
            for oi in range(4):
                m = const.tile([128, IB], BF16, tag=f"mask{oi}")
                nc.gpsimd.memset(m[:], 1.0)
                nc.gpsimd.affine_select(
                    out=m[:], in_=m[:],
                    compare_op=mybir.AluOpType.is_ge,
                    fill=0.0, base=-(oi * 128),
                    pattern=[[1, IB]], channel_multiplier=-1,
                )
                masks.append(m)

            ones_col = const.tile([128, NJ, 1], F32, tag="ones_col")
            nc.gpsimd.memset(ones_col[:], 1.0)

            # ---- weights resident in SBUF ----
            wq_sb = wpool.tile([128, 8, 3 * FL], MMDT, tag="w")
            nc.sync.dma_start(wq_sb[:], wqkvT_r)

            # ---- internal DRAM for the AllToAll ----
            a2a_in = dram.tile([N_CORES, FL, TSLICE], MMDT, tag="a2a_in")
            a2a_out = dram.tile([N_CORES, FL, TSLICE], MMDT,
                                tag="a2a_out")

            for b in range(B):
                # ---- qkv projection for batch b ----
                # q,k land transposed in qkvT[feat, tok]; v is staged per
                # token-block and PE-transposed straight into vt = [v | 1]
                qkvT = qkvpool.tile([128, 2, T], BF16, tag="qkvT")
                vts = []
                for hl in range(HL):
                    vt = vpool.tile([128, NJ, 128], BF16, tag="vt")
                    nc.vector.memset(vt[:, :, DH + 1:], 0.0)
                    nc.scalar.copy(vt[:, :, DH:DH + 1], ones_col[:])
                    vts.append(vt)
                for tb in range(T // IB):
                    xt = xpool.tile([128, 8, IB], MMDT, tag="xt")
                    t0 = b * T + tb * IB
                    nc.sync.dma_start(xt[:], xT_r[:, :, t0:t0 + IB])
                    for ft in range(3):
                        ps = psA.tile([128, IB], F32, tag="ps")
                        for dc in range(8):
                            nc.tensor.matmul(
                                ps[:],
                                lhsT=wq_sb[:, dc,
                                           ft * 128:(ft + 1) * 128],
                                rhs=xt[:, dc, :],
                                start=(dc == 0), stop=(dc == 7),
                            )
                        if ft < 2:
                            nc.vector.tensor_copy(
                                qkvT[:, ft, tb * IB:(tb + 1) * IB], ps[:])
                        else:
                            vst = xpool.tile([128, IB], MMDT, tag="vst")
                            nc.vector.tensor_copy(vst[:], ps[:])
                            for hl in range(HL):
                                for q4 in range(4):
                                    jt = tb * 4 + q4
                                    pst = psA.tile([128, IB], F32, tag="ps")
                                    nc.tensor.transpose(
                                        pst[:, :DH],
                                        vst[hl * DH:(hl + 1) * DH,
                                            q4 * 128:(q4 + 1) * 128
                                            ].bitcast(F32),
                                        identity[hl * DH:(hl + 1) * DH,
                                                 hl * DH:(hl + 1) * DH],
                                    )
                                    nc.scalar.copy(vts[hl][:, jt, :DH],
                                                   pst[:, :DH])

                # ---- causal attention per local head ----
                den_all = ypool.tile([2 * (T // IB), IB], F32,
                                     tag="den_all")
                recip_all = ypool.tile([2 * (T // IB), IB], F32,
                                       tag="recip_all")
                psy_keep = []
                for hl in range(HL):
                    vt = vts[hl]
                    q_sl = qkvT[hl * DH:(hl + 1) * DH, 0, :]
                    k_sl = qkvT[hl * DH:(hl + 1) * DH, 1, :]
                    for ib in range(T // IB):
                        nj = 4 * (ib + 1)
                        # phase S: all score matmuls back-to-back; exp+mask
                        # drain behind them on ACT/DVE into bf16 p tiles
                        ps_list = []
                        for jt in range(nj):
                            pss = psA.tile([128, IB], F32, tag="ps")
                            # transposed scores: [j(128), i(IB)]
                            nc.tensor.matmul(
                                pss[:],
                                lhsT=k_sl[:, jt * 128:(jt + 1) * 128],
                                rhs=q_sl[:, ib * IB:(ib + 1) * IB],
                                start=True, stop=True,
                            )
                            p = ppool.tile([128, IB], BF16, tag="p")
                            nc.scalar.activation(
                                p[:], pss[:],
                                mybir.ActivationFunctionType.Exp,
                                scale=SCALE,
                            )
                            if jt >= ib * 4:  # block-diagonal: apply mask
                                nc.vector.tensor_tensor(
                                    p[:], p[:], masks[jt - ib * 4][:],
                                    mybir.AluOpType.mult,
                                )
                            ps_list.append(p)
                        # phase PV: back-to-back accumulation into one bank
                        psy = psY.tile([128, IB], F32, tag="psy")
                        for jt in range(nj):
                            nc.tensor.matmul(
                                psy[:], lhsT=vt[:, jt, :], rhs=ps_list[jt][:],
                                start=(jt == 0), stop=(jt == nj - 1),
                            )
                        # stash denominator (row DH) and drain y to SBUF
                        r = hl * (T // IB) + ib
                        den_sb = ypool.tile([1, IB], F32, tag="den_sb")
                        nc.scalar.copy(den_sb[:], psy[DH:DH + 1, :])
                        nc.sync.dma_start(den_all[r:r + 1, :], den_sb[:])
                        y_un = yun.tile([DH, IB], F32, tag="y_un")
                        nc.vector.tensor_copy(y_un[:], psy[:DH, :])
                        psy_keep.append((hl, ib, y_un))
                # one batched reciprocal for all 8 denominator rows
                nc.vector.reciprocal(recip_all[:], den_all[:])
                for hl, ib, y_un in psy_keep:
                    r = hl * (T // IB) + ib
                    recip_sb = ypool.tile([1, IB], F32, tag="recip_sb")
                    nc.sync.dma_start(recip_sb[:], recip_all[r:r + 1, :])
                    rb = ypool.tile([DH, IB], F32, tag="rb")
                    nc.gpsimd.partition_broadcast(rb[:], recip_sb[:])
                    ysb = ypool.tile([DH, IB], MMDT, tag="ysb")
                    nc.vector.tensor_tensor(ysb[:], y_un[:], rb[:],
                                            mybir.AluOpType.mult)
                    g0 = b * T + ib * IB
                    nc.sync.dma_start(
                        a2a_in[g0 // TSLICE, hl * DH:(hl + 1) * DH,
                               (g0 % TSLICE):(g0 % TSLICE) + IB],
                        ysb[:],
                    )

            # w_out replaces w_qkv in the shared weight slot once the
            # last projection matmul has read wq_sb
            wout_sb = wpool.tile([128, 8, D], MMDT, tag="w")
            nc.sync.dma_start(wout_sb[:], woutT_r)

            # ---- reshard y: head-split -> token-split ----
            nc.gpsimd.collective_compute(
                "AllToAll", mybir.AluOpType.bypass,
                replica_groups=[list(range(N_CORES))],
                ins=[a2a_in[:]], outs=[a2a_out[:]],
            )
            a2a_out_r = a2a_out[:].rearrange("f p t -> p f t")  # [128,8,1024]

            # ---- output projection for this core's token slice ----
            for tt in range(TSLICE // 128):
                lh = opool.tile([128, 8, 128], MMDT, tag="lh")
                nc.sync.dma_start(lh[:],
                                  a2a_out_r[:, :, tt * 128:(tt + 1) * 128])
                for db in range(D // IB):
                    pso = psA.tile([128, IB], F32, tag="ps")
                    for fc in range(8):
                        nc.tensor.matmul(
                            pso[:], lhsT=lh[:, fc, :],
                            rhs=wout_sb[:, fc, db * IB:(db + 1) * IB],
                            start=(fc == 0), stop=(fc == 7),
                        )
                    osb = opool.tile([128, IB], F32, tag="osb")
                    nc.scalar.copy(osb[:], pso[:])
                    nc.sync.dma_start(
                        out[tt * 128:(tt + 1) * 128, db * IB:(db + 1) * IB],
                        osb[:],
                    )

    nc.finalize()
    return nc


_NC_CACHE = {}


def _get_nc():
    if "nc" not in _NC_CACHE:
        _NC_CACHE["nc"] = _build()
    return _NC_CACHE["nc"]


def kernel(x, w_qkv, w_out):
    x = np.asarray(x, np.float32).reshape(BT, D)
    w_qkv = np.asarray(w_qkv, np.float32)
    w_out = np.asarray(w_out, np.float32)

    xT = np.ascontiguousarray(x.T)
    woutT = np.ascontiguousarray(w_out.T)

    in_maps = []
    for c in range(N_CORES):
        rows = []
        for t in range(3):
            for hl in range(HL):
                h = HL * c + hl
                rows.append(w_qkv[t * H * DH + h * DH:
                                  t * H * DH + (h + 1) * DH])
        wq_c = np.concatenate(rows, axis=0)  # [384, D]
        in_maps.append({
            "xT": xT,
            "wqkvT": np.ascontiguousarray(wq_c.T),
            "woutT": woutT,
        })

    nc = _get_nc()
    do_trace = bool(os.environ.get("ATTN_TRACE"))
    if do_trace:
        _install_profile_hook()
    res = run_bass_kernel_spmd(nc, in_maps, list(range(N_CORES)),
                               trace=do_trace)
    if res.exec_time_ns is not None:
        print(f"HW exec time: {res.exec_time_ns} ns")
        _NC_CACHE["exec_time_ns"] = res.exec_time_ns
        _NC_CACHE["trace"] = res.instructions_and_trace
    full = np.concatenate([res.results[c]["out"] for c in range(N_CORES)],
                          axis=0)
    return full.reshape(B, T, D)


# revision 15
# speedup vs baseline: 1.3311x; 1.3311x over previous
"""Distributed causal self-attention on 8 TRN2 NeuronCores.

Strategy (Megatron-style tensor parallel on heads + token-split output):
  - Each core owns 2 of the 16 heads. It computes qkv projection for its
    heads (full batch/seq), causal attention (transposed-score layout so
    softmax sums come free from an appended ones-row on V), producing
    y[heads_local, tokens].
  - AllToAll reshards y from head-split to token-split: afterwards each
    core holds all 1024 features for its 1024-token slice.
  - Each core computes its token slice of the output projection.
  - Host gather is pure concatenation along tokens.

Layouts: everything TensorEngine-facing keeps the contraction dim on
partitions. x is passed pre-transposed [D, B*T]; qkv weights pre-sliced
and transposed per core [D, 384]; w_out pre-transposed [feat, D].
"""

import os
import sys

sys.path.insert(0, "/opt/trn_rl_repo")

import numpy as np

import concourse.bass as bass
import concourse.mybir as mybir
import concourse.tile as tile
from concourse import bacc
from concourse.bass_utils import run_bass_kernel_spmd
from concourse.masks import make_identity

def _install_profile_hook():
    """The RL container's antenv stub lacks axon_hooks, so bass_utils can't
    reach the NTFF profiler. Recreate the tiny set/get module and wire it to
    trn_boot's ctypes hook against libaxon_pjrt.so."""
    import types

    if "antenv.axon_hooks" in sys.modules:
        return
    try:
        import antenv
        from trn_agent_boot.trn_boot import _ntff_profile_via_ctypes

        mod = types.ModuleType("antenv.axon_hooks")
        mod._hook = None

        def set_axon_ntff_profile_hook(h):
            mod._hook = h

        def get_axon_ntff_profile_hook():
            return mod._hook

        mod.set_axon_ntff_profile_hook = set_axon_ntff_profile_hook
        mod.get_axon_ntff_profile_hook = get_axon_ntff_profile_hook
        sys.modules["antenv.axon_hooks"] = mod
        antenv.axon_hooks = mod
        hook = _ntff_profile_via_ctypes("/opt/axon/libaxon_pjrt.so")
        if hook is not None:
            mod._hook = hook
    except Exception as e:  # profiling is best-effort; execution must work
        print(f"profile hook install failed: {e}", file=sys.stderr)


B, T, D, H, DH = 4, 2048, 1024, 16, 64
BT = B * T              # 8192 tokens
N_CORES = 8
HL = H // N_CORES       # 2 heads per core
FL = HL * DH            # 128 local features
TSLICE = BT // N_CORES  # 1024 tokens per core in output
SCALE = DH ** -0.5
F32 = mybir.dt.float32
# TensorEngine compute dtype. float32r is the same 4-byte memory format but
# runs 1.5 cycles/row on the PE instead of fp32's 2.
USE_F32R = os.environ.get("ATTN_F32R", "1") != "0"
F32R = mybir.dt.float32r


MMDT = F32R if USE_F32R else F32
BF16 = mybir.dt.bfloat16

IB = 512     # query block (free dim of transposed score matmuls)
NJ = T // 128  # 16 key tiles per (b, h)


def _build():
    nc = bacc.Bacc("TRN2", target_bir_lowering=False, debug=False,
                   num_devices=N_CORES)

    xT = nc.dram_tensor("xT", [D, BT], MMDT, kind="ExternalInput")
    wqkvT = nc.dram_tensor("wqkvT", [D, 3 * FL], MMDT,
                           kind="ExternalInput")
    woutT = nc.dram_tensor("woutT", [D, D], MMDT, kind="ExternalInput")
    out = nc.dram_tensor("out", [TSLICE, D], F32, kind="ExternalOutput")

    xT_r = xT[:].rearrange("(o p) t -> p o t", p=128)        # [128, 8, BT]
    wqkvT_r = wqkvT[:].rearrange("(o p) f -> p o f", p=128)  # [128, 8, 384]
    woutT_r = woutT[:].rearrange("(o p) d -> p o d", p=128)  # [128, 8, 1024]

    with tile.TileContext(nc) as tc:
        from contextlib import ExitStack

        with ExitStack() as ctx:
            const = ctx.enter_context(tc.tile_pool(name="const", bufs=1))
            wpool = ctx.enter_context(tc.tile_pool(name="wpool", bufs=1))
            xpool = ctx.enter_context(tc.tile_pool(name="xpool", bufs=2))
            qkvpool = ctx.enter_context(tc.tile_pool(name="qkvpool", bufs=2))
            yun = ctx.enter_context(tc.tile_pool(name="yun", bufs=9))
            vpool = ctx.enter_context(tc.tile_pool(name="vpool", bufs=3))
            ppool = ctx.enter_context(tc.tile_pool(name="ppool", bufs=20))
            ypool = ctx.enter_context(tc.tile_pool(name="ypool", bufs=2))
            opool = ctx.enter_context(tc.tile_pool(name="opool", bufs=2))
            psA = ctx.enter_context(
                tc.tile_pool(name="psA", bufs=5, space="PSUM"))
            psY = ctx.enter_context(
                tc.tile_pool(name="psY", bufs=2, space="PSUM"))
            dram = ctx.enter_context(
                tc.tile_pool(name="dram", bufs=1, space="DRAM"))

            # ---- constants ----
            identity = const.tile([128, 128], F32, tag="identity")
            make_identity(nc, identity[:])
            # masks[oi][p, f] = 1.0 where f - p - oi*128 >= 0 else 0
            # (keep key j0+p for query i0+f iff j <= i, offset oi*128 = j0-i0)
            masks = []
            for oi in range(4):
                m = const.tile([128, IB], BF16, tag=f"mask{oi}")
                nc.gpsimd.memset(m[:], 1.0)
                nc.gpsimd.affine_select(
                    out=m[:], in_=m[:],
                    compare_op=mybir.AluOpType.is_ge,
                    fill=0.0, base=-(oi * 128),
                    pattern=[[1, IB]], channel_multiplier=-1,
                )
                masks.append(m)

            ones_col = const.tile([128, NJ, 1], F32, tag="ones_col")
            nc.gpsimd.memset(ones_col[:], 1.0)

            # ---- weights resident in SBUF ----
            wq_sb = wpool.tile([128, 8, 3 * FL], MMDT, tag="w")
            nc.sync.dma_start(wq_sb[:], wqkvT_r)

            # ---- internal DRAM for the AllToAll ----
            a2a_in = dram.tile([N_CORES, FL, TSLICE], MMDT, tag="a2a_in")
            a2a_out = dram.tile([N_CORES, FL, TSLICE], MMDT,
                                tag="a2a_out")

            for b in range(B):
                # ---- qkv projection for batch b ----
                # q,k land transposed in qkvT[feat, tok]; v is staged per
                # token-block and PE-transposed straight into vt = [v | 1]
                qkvT = qkvpool.tile([128, 4, T], BF16, tag="qkvT")
                # zero the K-padding rows so padded scores contract cleanly
                nc.vector.memset(qkvT[64:128, 0, :], 0.0)
                nc.vector.memset(qkvT[0:64, 1, :], 0.0)
                nc.vector.memset(qkvT[64:128, 2, :], 0.0)
                nc.vector.memset(qkvT[0:64, 3, :], 0.0)
                vts = []
                for hl in range(HL):
                    vt = vpool.tile([128, NJ, 128], BF16, tag="vt")
                    nc.vector.memset(vt[:, :, DH + 1:], 0.0)
                    nc.scalar.copy(vt[:, :, DH:DH + 1], ones_col[:])
                    vts.append(vt)
                for tb in range(T // IB):
                    xt = xpool.tile([128, 8, IB], MMDT, tag="xt")
                    t0 = b * T + tb * IB
                    nc.sync.dma_start(xt[:], xT_r[:, :, t0:t0 + IB])
                    for ft in range(3):
                        ps = psA.tile([128, IB], F32, tag="ps")
                        for dc in range(8):
                            nc.tensor.matmul(
                                ps[:],
                                lhsT=wq_sb[:, dc,
                                           ft * 128:(ft + 1) * 128],
                                rhs=xt[:, dc, :],
                                start=(dc == 0), stop=(dc == 7),
                            )
                        if ft < 2:
                            sl = tb * IB
                            nc.vector.tensor_copy(
                                qkvT[0:64, 2 * ft, sl:sl + IB], ps[0:64, :])
                            nc.vector.tensor_copy(
                                qkvT[64:128, 2 * ft + 1, sl:sl + IB],
                                ps[64:128, :])
                        else:
                            vst = xpool.tile([128, IB], MMDT, tag="vst")
                            nc.vector.tensor_copy(vst[:], ps[:])
                            for hl in range(HL):
                                for q4 in range(4):
                                    jt = tb * 4 + q4
                                    pst = psA.tile([128, IB], F32, tag="ps")
                                    nc.tensor.transpose(
                                        pst[:, :DH],
                                        vst[hl * DH:(hl + 1) * DH,
                                            q4 * 128:(q4 + 1) * 128
                                            ].bitcast(F32),
                                        identity[hl * DH:(hl + 1) * DH,
                                                 hl * DH:(hl + 1) * DH],
                                    )
                                    nc.scalar.copy(vts[hl][:, jt, :DH],
                                                   pst[:, :DH])

                # ---- causal attention per local head ----
                den_all = ypool.tile([2 * (T // IB), IB], F32,
                                     tag="den_all")
                recip_all = ypool.tile([2 * (T // IB), IB], F32,
                                       tag="recip_all")
                psy_keep = []
                for hl in range(HL):
                    vt = vts[hl]
                    q_sl = qkvT[:, hl, :]
                    k_sl = qkvT[:, 2 + hl, :]
                    for ib in range(T // IB):
                        nj = 4 * (ib + 1)
                        # phase S: all score matmuls back-to-back; exp+mask
                        # drain behind them on ACT/DVE into bf16 p tiles
                        ps_list = []
                        for jt in range(nj):
                            pss = psA.tile([128, IB], F32, tag="ps")
                            # transposed scores: [j(128), i(IB)]
                            nc.tensor.matmul(
                                pss[:],
                                lhsT=k_sl[:, jt * 128:(jt + 1) * 128],
                                rhs=q_sl[:, ib * IB:(ib + 1) * IB],
                                start=True, stop=True,
                            )
                            p = ppool.tile([128, IB], BF16, tag="p")
                            nc.scalar.activation(
                                p[:], pss[:],
                                mybir.ActivationFunctionType.Exp,
                                scale=SCALE,
                            )
                            if jt >= ib * 4:  # block-diagonal: apply mask
                                nc.vector.tensor_tensor(
                                    p[:], p[:], masks[jt - ib * 4][:],
                                    mybir.AluOpType.mult,
                                )
                            ps_list.append(p)
                        # phase PV: back-to-back accumulation into one bank
                        psy = psY.tile([128, IB], F32, tag="psy")
                        for jt in range(nj):
                            nc.tensor.matmul(
                                psy[:], lhsT=vt[:, jt, :], rhs=ps_list[jt][:],
                                start=(jt == 0), stop=(jt == nj - 1),
                            )
                        # stash denominator (row DH) and drain y to SBUF
                        r = hl * (T // IB) + ib
                        den_sb = ypool.tile([1, IB], F32, tag="den_sb")
                        nc.scalar.copy(den_sb[:], psy[DH:DH + 1, :])
                        nc.sync.dma_start(den_all[r:r + 1, :], den_sb[:])
                        y_un = yun.tile([DH, IB], F32, tag="y_un")
                        nc.vector.tensor_copy(y_un[:], psy[:DH, :])
                        psy_keep.append((hl, ib, y_un))
                # one batched reciprocal for all 8 denominator rows
                nc.vector.reciprocal(recip_all[:], den_all[:])
                for hl, ib, y_un in psy_keep:
                    r = hl * (T // IB) + ib
                    recip_sb = ypool.tile([1, IB], F32, tag="recip_sb")
                    nc.sync.dma_start(recip_sb[:], recip_all[r:r + 1, :])
                    rb = ypool.tile([DH, IB], F32, tag="rb")
                    nc.gpsimd.partition_broadcast(rb[:], recip_sb[:])
                    ysb = ypool.tile([DH, IB], MMDT, tag="ysb")
                    nc.vector.tensor_tensor(ysb[:], y_un[:], rb[:],
                                            mybir.AluOpType.mult)
                    g0 = b * T + ib * IB
                    nc.sync.dma_start(
                        a2a_in[g0 // TSLICE, hl * DH:(hl + 1) * DH,
                               (g0 % TSLICE):(g0 % TSLICE) + IB],
                        ysb[:],
                    )

            # w_out replaces w_qkv in the shared weight slot once the
            # last projection matmul has read wq_sb
            wout_sb = wpool.tile([128, 8, D], MMDT, tag="w")
            nc.sync.dma_start(wout_sb[:], woutT_r)

            # ---- reshard y: head-split -> token-split ----
            nc.gpsimd.collective_compute(
                "AllToAll", mybir.AluOpType.bypass,
                replica_groups=[list(range(N_CORES))],
                ins=[a2a_in[:]], outs=[a2a_out[:]],
            )
            a2a_out_r = a2a_out[:].rearrange("f p t -> p f t")  # [128,8,1024]

            # ---- output projection for this core's token slice ----
            for tt in range(TSLICE // 128):
                lh = opool.tile([128, 8, 128], MMDT, tag="lh")
                nc.sync.dma_start(lh[:],
                                  a2a_out_r[:, :, tt * 128:(tt + 1) * 128])
                for db in range(D // IB):
                    pso = psA.tile([128, IB], F32, tag="ps")
                    for fc in range(8):
                        nc.tensor.matmul(
                            pso[:], lhsT=lh[:, fc, :],
                            rhs=wout_sb[:, fc, db * IB:(db + 1) * IB],
                            start=(fc == 0), stop=(fc == 7),
                        )
                    osb = opool.tile([128, IB], F32, tag="osb")
                    nc.scalar.copy(osb[:], pso[:])
                    nc.sync.dma_start(
                        out[tt * 128:(tt + 1) * 128, db * IB:(db + 1) * IB],
                        osb[:],
                    )

    nc.finalize()
    return nc


_NC_CACHE = {}


def _get_nc():
    if "nc" not in _NC_CACHE:
        _NC_CACHE["nc"] = _build()
    return _NC_CACHE["nc"]


def kernel(x, w_qkv, w_out):
    x = np.asarray(x, np.float32).reshape(BT, D)
    w_qkv = np.asarray(w_qkv, np.float32)
    w_out = np.asarray(w_out, np.float32)

    xT = np.ascontiguousarray(x.T)
    woutT = np.ascontiguousarray(w_out.T)

    in_maps = []
    for c in range(N_CORES):
        rows = []
        for t in range(3):
            for hl in range(HL):
                h = HL * c + hl
                rows.append(w_qkv[t * H * DH + h * DH:
                                  t * H * DH + (h + 1) * DH])
        wq_c = np.concatenate(rows, axis=0)  # [384, D]
        in_maps.append({
            "xT": xT,
            "wqkvT": np.ascontiguousarray(wq_c.T),
            "woutT": woutT,
        })

    nc = _get_nc()
    do_trace = bool(os.environ.get("ATTN_TRACE"))
    if do_trace:
        _install_profile_hook()
    res = run_bass_kernel_spmd(nc, in_maps, list(range(N_CORES)),
                               trace=do_trace)
    if res.exec_time_ns is not None:
        print(f"HW exec time: {res.exec_time_ns} ns")
        _NC_CACHE["exec_time_ns"] = res.exec_time_ns
        _NC_CACHE["trace"] = res.instructions_and_trace
    full = np.concatenate([res.results[c]["out"] for c in range(N_CORES)],
                          axis=0)
    return full.reshape(B, T, D)
